# revision 13
# baseline (speedup 1.0000x reference)
"""Single-head causal attention on 8 Trainium2 NeuronCores.

Problem: x:[8,2048,1024], Wq/Wk/Wv:[64,1024], bq/bk/bv:[64]
  q,k,v = x@W*.T + b*;  out = softmax(causal(q@k.T)/sqrt(64)) @ v

Sharding: batch dim (8) across the 8 cores - fully data-parallel, no
collectives. Each core computes one batch's attention head.

v2 design (all matmuls bf16, 1 cycle/row on the PE at any free size):
  - x and the packed projection weights are converted to bf16 on the host:
    halves HBM traffic and frees the fp32r >=256-free-dim constraint.
  - k/v projection stays h-major ([Wk|Wv] packed stationary, x moving:
    full 128-wide output). q projection runs s-major: x tiles double as
    stationary [128e,128s] blocks, Wq.T [128e,64] is the moving operand -
    64 rows per (e,s) pair instead of 512 per (e,chunk), 2x fewer PE rows
    for q. q then transposes back to h-major on the PE (bf16 identity).
  - causal dead-column skipping: for diagonal k-tiles only columns
    [128i, 512) are live; scores/exp/AV all operate on the live slice
    (saves ~2.5us PE + ~2.5us ACT). Only the [128,128] triangle block
    needs the 0/1 mask multiply (DVE, bf16).
  - ACT runs exp almost exclusively (the bottleneck engine in v1 at 82%);
    q/k epilogue copies live on DVE, v on ACT, outputs DMA straight from
    PSUM (no SBUF staging).
  - softmax without max-subtraction (scores/8 ~ N(0,1); exp safe in f32),
    denominator = ones-row appended to V, division on the host.
  - emission interleaves proj(c+1) with attn(c); the last chunk processes
    its (cheap, live-sliced) diagonal tiles first so the drain is fed by
    exps computed well in advance.
"""

import numpy as np
import ml_dtypes

import concourse.bacc as bacc
import concourse.mybir as mybir
import concourse.tile as tile
from concourse import bass2jax

B, S, E, H = 8, 2048, 1024, 64
NCORES = 8
PB = 128  # partition block / k-tile size
QB = 512  # q-chunk (matmul moving free dim)
ET = E // PB  # e-tiles per contraction
QC = S // QB  # q-chunks
KT = S // PB  # k-tiles
DIAG = QB // PB  # diagonal k-tiles per q-chunk

# packed bf16 constants layout: columns of the [128, NCONST] "consts" input
C_ID = 0  # [*, 0:128]    identity 128x128
C_TRI = 128  # [*, 128:256]  causal triangle: (p, j) = 1 iff j >= p
C_ONES = 256  # [*, 256:257]  ones
NCONST = 257
# f32 bias constants: columns of the [128, NBIAS] "biases" input
CB_K = 0  # k bias (rows 0:64)
CB_V = 1  # v bias (rows 64:128)
CB_Q = 2  # q bias / 8 (rows 0:64)
NBIAS = 3

F32 = mybir.dt.float32
BF16 = mybir.dt.bfloat16
AF = mybir.ActivationFunctionType
MUL = mybir.AluOpType.mult
ADD = mybir.AluOpType.add

_CACHE: dict = {}

# schedule/buffering knobs (sweepable)
CFG = {
    "lookahead": 7,
    "xbufs": 12,
    "wtbufs": 10,
    "psbufs": 3,
    "dma2": "gpsimd",  # second x-stream queue
    "cs_q": "gpsimd",
    "diag_first": True,
}


def _interleave(*gens):
    """Drive generators round-robin; the first (proj) gets two steps per turn."""
    alive = list(gens)
    steps = {id(g): (2 if i == 0 and len(gens) > 1 else 1) for i, g in enumerate(gens)}
    while alive:
        for g in list(alive):
            for _ in range(steps[id(g)]):
                try:
                    next(g)
                except StopIteration:
                    alive.remove(g)
                    break


def _build_nc():
    nc = bacc.Bacc("TRN2", target_bir_lowering=False, debug=False)
    xT = nc.dram_tensor("xT", [E, S], BF16, kind="ExternalInput").ap()
    # cols 0:64 Wk.T, 64:128 Wv.T, 128:192 Wq.T
    wqkv = nc.dram_tensor("wqkv", [E, 3 * H], BF16, kind="ExternalInput").ap()
    consts = nc.dram_tensor("consts", [PB, NCONST], BF16, kind="ExternalInput").ap()
    biases = nc.dram_tensor("biases", [PB, NBIAS], F32, kind="ExternalInput").ap()
    out = nc.dram_tensor("out", [H + 1, S], F32, kind="ExternalOutput").ap()

    with tile.TileContext(nc) as tc:
        with (
            tc.tile_pool(name="const", bufs=1) as constp,
            tc.tile_pool(name="xs", bufs=CFG["xbufs"]) as xpool,
            tc.tile_pool(name="qkv", bufs=1) as qkvp,
            tc.tile_pool(name="qsm", bufs=2) as qsmp,
            tc.tile_pool(name="wt", bufs=CFG["wtbufs"]) as wtp,
            tc.tile_pool(name="fin", bufs=2) as finp,
            tc.tile_pool(name="pkv", bufs=2, space="PSUM") as pvp,
            tc.tile_pool(name="pq", bufs=2, space="PSUM") as pqp,
            tc.tile_pool(name="ps", bufs=CFG["psbufs"], space="PSUM") as psp,
            tc.tile_pool(name="pav", bufs=1, space="PSUM") as pavp,
        ):
            # wqkv split in two: the e=0/1 slice unblocks the first projection
            # matmul earlier than one big transfer.
            wqkv_sb = constp.tile([PB, ET, 3 * H], BF16)
            nc.scalar.dma_start(
                wqkv_sb[:, 0:2, :],
                wqkv[0 : 2 * PB, :].rearrange("(t p) m -> p t m", p=PB),
            )
            nc.gpsimd.dma_start(
                wqkv_sb[:, 2:ET, :],
                wqkv[2 * PB :, :].rearrange("(t p) m -> p t m", p=PB),
            )
            cs = constp.tile([PB, NCONST], BF16)
            bs = constp.tile([PB, NBIAS], F32)

            id128_ap = cs[:, C_ID : C_ID + PB]
            idv_ap = cs[H:PB, C_ID + H : C_ID + PB]  # eye(64) at partitions 64:128
            tri_ap = cs[:, C_TRI : C_TRI + PB]
            kb_ap = bs[0:H, CB_K : CB_K + 1]
            vb_ap = bs[H:PB, CB_V : CB_V + 1]
            qb_ap = bs[0:H, CB_Q : CB_Q + 1]
            ones_ap = cs[:, C_ONES : C_ONES + 1]

            qT = qkvp.tile([H, S], BF16)  # q/8 h-major
            kT = qkvp.tile([H, S], BF16)  # k h-major
            vTh = qkvp.tile([PB, S], BF16)  # v h-major at partitions 64:128
            vsb = qkvp.tile([PB, KT, H + 1], BF16)  # v k-major + ones col

            def load_consts():
                # emitted after chunk 0's x tiles so the pool queue's first
                # deliveries are the tiles the first accumulation needs
                getattr(nc, CFG["cs_q"]).dma_start(cs[:], consts[:])
                getattr(nc, CFG["cs_q"]).dma_start(bs[:], biases[:])
                nc.vector.tensor_copy(
                    vsb[:, :, H : H + 1],
                    ones_ap[:, 0:1, None].to_broadcast((PB, KT, 1)),
                )

            proj_state = {}

            def proj_main(c):
                # DMA + accumulating matmuls only; epilogue emitted separately
                # (proj_epi) after the previous chunk's attention.
                qs = slice(c * QB, (c + 1) * QB)
                p_kv = pvp.tile([PB, QB], F32, tag="pkv")
                # q s-major; full-bank tile so its start's zero region (the
                # whole 2KB PSUM bank) can't clobber a co-resident tile
                p_q = pqp.tile([PB, QB], F32, tag="pq")
                proj_state[c] = (p_kv, p_q)
                for e in range(ET):
                    xt = xpool.tile([PB, QB], BF16, tag="xt")
                    if c == 0:
                        # three-way split shortens the startup-critical load
                        dma_eng = (nc.sync, nc.scalar, getattr(nc, CFG["dma2"]))[e % 3]
                    else:
                        dma_eng = nc.sync if e % 2 == 0 else getattr(nc, CFG["dma2"])
                    dma_eng.dma_start(xt[:], xT[e * PB : (e + 1) * PB, qs])
                    nc.tensor.matmul(
                        p_kv[:],
                        wqkv_sb[:, e, 0:PB],
                        xt[:],
                        start=(e == 0),
                        stop=(e == ET - 1),
                    )
                    # q s-major: x block as stationary, Wq.T as moving. Four
                    # accumulation groups share one PSUM bank: only the very
                    # first matmul may set start (start zeroes the whole
                    # bank); groups 1-3's first writes land on still-pending
                    # zero bytes and store rather than accumulate.
                    for j in range(DIAG):
                        nc.tensor.matmul(
                            p_q[:, j * H : (j + 1) * H],
                            xt[:, j * PB : (j + 1) * PB],
                            wqkv_sb[:, e, 2 * H : 3 * H],
                            start=(e == 0 and j == 0),
                            stop=(e == ET - 1 and j == DIAG - 1),
                            skip_group_check=True,
                        )
                    yield

            def proj_epi(c):
                qs = slice(c * QB, (c + 1) * QB)
                p_kv, p_q = proj_state.pop(c)
                # k on DVE (gates scores), v on ACT
                nc.vector.tensor_scalar(
                    kT[:, qs], p_kv[0:H, :], kb_ap, None, ADD, mybir.AluOpType.bypass
                )
                nc.scalar.activation(
                    vTh[H:PB, qs], p_kv[H:PB, :], AF.Identity, bias=vb_ap
                )
                # q: PSUM s-major -> SBUF -> PE transpose -> h-major (scale+bias)
                qsm = qsmp.tile([PB, DIAG * H], BF16, tag="qsm")
                nc.vector.tensor_copy(qsm[:], p_q[:, 0 : DIAG * H])
                for j in range(DIAG):
                    p_qt = pqp.tile([H, PB], BF16, tag="pq")
                    nc.tensor.transpose(p_qt[:], qsm[:, j * H : (j + 1) * H], id128_ap)
                    nc.vector.tensor_scalar(
                        qT[:, c * QB + j * PB : c * QB + (j + 1) * PB],
                        p_qt[:],
                        0.125,
                        qb_ap,
                        MUL,
                        ADD,
                    )
                for t in range(DIAG):
                    m = DIAG * c + t
                    p_vt = pvp.tile([PB, H], BF16, tag="pkv")
                    nc.tensor.transpose(
                        p_vt[:], vTh[H:PB, m * PB : (m + 1) * PB], idv_ap
                    )
                    nc.vector.tensor_copy(vsb[:, m, 0:H], p_vt[:])

            def attn(c):
                nkt = DIAG * c + DIAG
                p_av = pavp.tile([H + 1, QB], F32, tag="pav")

                def live_lo(m):
                    i = m - DIAG * c
                    return i * PB if i > 0 else 0

                def weights_tile(m):
                    # scores -> exp -> (diagonal) causal triangle mask,
                    # live columns only
                    lo = live_lo(m)
                    p_s = psp.tile([PB, QB], F32, tag="ps")
                    nc.tensor.matmul(
                        p_s[:, lo:QB],
                        kT[:, m * PB : (m + 1) * PB],
                        qT[:, c * QB + lo : (c + 1) * QB],
                        start=True,
                        stop=True,
                    )
                    w = wtp.tile([PB, QB], BF16, tag="w")
                    nc.scalar.activation(w[:, lo:QB], p_s[:, lo:QB], AF.Exp)
                    i = m - DIAG * c
                    if i >= 0:
                        nc.vector.tensor_tensor(
                            w[:, lo : lo + PB], w[:, lo : lo + PB], tri_ap, MUL
                        )
                    return w

                L = CFG["lookahead"]
                if c == QC - 1 and CFG["diag_first"]:
                    # final chunk: diagonals first so the drain of the last
                    # (unpipelined) m-steps has no exp->mask->AV chain
                    order = list(range(DIAG * c, nkt)) + list(range(0, DIAG * c))
                else:
                    order = list(range(nkt))
                ws = {m: weights_tile(m) for m in order[: min(L, nkt)]}
                yield
                for idx, m in enumerate(order):
                    if idx + L < nkt:
                        ws[order[idx + L]] = weights_tile(order[idx + L])
                    lo = live_lo(m)
                    nc.tensor.matmul(
                        p_av[:, lo:QB],
                        vsb[:, m, :],
                        ws.pop(m)[:, lo:QB],
                        start=(idx == 0),
                        stop=(idx == nkt - 1),
                    )
                    yield
                # unnormalized output + denominator row; division happens on
                # the host as part of unsharding. The last chunk goes out in
                # column halves so half 2's copy overlaps half 1's DMA.
                osb = finp.tile([H + 1, QB], F32, tag="osb")
                if c == QC - 1:
                    hw_ = QB // 2
                    for hh in range(2):
                        cols = slice(hh * hw_, (hh + 1) * hw_)
                        nc.vector.tensor_copy(osb[:, cols], p_av[:, cols])
                        oq = nc.sync if hh == 0 else nc.scalar
                        oq.dma_start(
                            out[:, c * QB + hh * hw_ : c * QB + (hh + 1) * hw_],
                            osb[:, cols],
                        )
                        yield
                else:
                    nc.vector.tensor_copy(osb[:], p_av[:])
                    yield
                    nc.sync.dma_start(out[:, c * QB : (c + 1) * QB], osb[:])
                    yield

            # interleaved emission: proj_main(c) alternates with attn(c-1) so
            # the in-order engine queues see attention work during DMA waits;
            # each projection epilogue is emitted after that attention so no
            # exp/mask queues behind an epilogue copy still waiting on DMA.
            g0 = proj_main(0)
            for _ in range(4):
                next(g0)  # chunk 0's first x tiles lead both DMA queues
            load_consts()
            _interleave(g0)
            proj_epi(0)
            for c in range(1, QC):
                _interleave(proj_main(c), attn(c - 1))
                proj_epi(c)
            _interleave(attn(QC - 1))

    nc.compile()
    return nc


def _host_inputs(x, Wq, bq, Wk, bk, Wv, bv):
    bf16 = ml_dtypes.bfloat16
    x = np.asarray(x, np.float32)
    Wq, bq = np.asarray(Wq, np.float32), np.asarray(bq, np.float32)
    Wk, bk = np.asarray(Wk, np.float32), np.asarray(bk, np.float32)
    Wv, bv = np.asarray(Wv, np.float32), np.asarray(bv, np.float32)

    wqkv = np.ascontiguousarray(
        np.concatenate([Wk.T, Wv.T, Wq.T], axis=1)
    ).astype(bf16)  # [E, 3H]

    cs = np.zeros((PB, NCONST), np.float32)
    cs[:, C_ID : C_ID + PB] = np.eye(PB, dtype=np.float32)
    jj = np.arange(PB, dtype=np.int64)[None, :]
    pp = np.arange(PB, dtype=np.int64)[:, None]
    cs[:, C_TRI : C_TRI + PB] = (jj >= pp).astype(np.float32)
    cs[:, C_ONES] = 1.0
    cs = cs.astype(bf16)

    bsc = np.zeros((PB, NBIAS), np.float32)
    bsc[:H, CB_K] = bk
    bsc[H:PB, CB_V] = bv
    bsc[:H, CB_Q] = bq * 0.125

    shared = {"wqkv": wqkv, "consts": cs, "biases": bsc}
    in_maps = []
    for b in range(B):
        m = dict(shared)
        m["xT"] = np.ascontiguousarray(x[b].T).astype(bf16)
        in_maps.append(m)
    return in_maps


def get_nc():
    if "nc" not in _CACHE:
        _CACHE["nc"] = _build_nc()
    return _CACHE["nc"]


def kernel(x, Wq, bq, Wk, bk, Wv, bv):
    nc = get_nc()
    in_maps = _host_inputs(x, Wq, bq, Wk, bk, Wv, bv)
    results = bass2jax.run_bass_via_pjrt(nc, in_maps, n_cores=NCORES)
    out = np.empty((B, S, H), np.float32)
    for b in range(B):
        o = results[b]["out"]
        out[b] = (o[:H] / o[H : H + 1]).T
    return out
